# revision 7
# baseline (speedup 1.0000x reference)
"""CTLSTM cell fused kernel for 8 Trainium2 NeuronCores.

Strategy (data-parallel over batch, weight-stationary transposed matmul):
  - B=16384 rows sharded 2048/core; weights replicated.
  - TRANSPOSED layout: gate columns live on PSUM partitions, batch on the
    moving/free dim.  out[g_chunk, b_tile] = w2[k, g_chunk].T @ xh[k, b_tile].
    The stationary operand is the weight block [128,128], reused across 4
    consecutive matmuls (2048 moving batch columns) -> back-to-back 216ns
    matmuls on the PE (vs 259ns with per-matmul weight swaps).
  - Host stages xh = [x;ht].T as [8, 128, 2048] bf16 k-chunks and
    w2 = [Wx;Wh].T (gate-permuted) as [8, 128, 3584] bf16.
  - Gate rows are host-permuted j-major: for each 128-row h-chunk j, the 7
    gate chunks [z, i, f, d, ib, fb, o] for that j are adjacent.  That
    order lets the elementwise work drain mid-j: c (needs z,i,f) right
    after f, decay_rate after d, c_bar after fb; only h (needs o) runs
    after the j's last matmul, so the post-PE tail is tiny.
  - Bias is per-PARTITION in this layout: every PSUM drain is a single ACT
    op func(psum + bias) straight into SBUF (bias for the d-gate is
    negated: drain computes sigmoid(-(wd)) for the softplus trick).
    No DVE bias-adds, no [128, G] broadcast bias tile.
  - softplus(wd) = -ln(sigmoid(-wd)); Ln runs batched once per j (2 ACT
    table switches per j, hidden under PE work), then DVE negates.
  - DMA: input loads issue from the idle GpSimd sequencer (fast dispatch),
    interleaved per k-chunk so the PE starts accumulating k=0 while k=1..
    stream in; stores issue from SP.
"""

import numpy as np
import ml_dtypes

import concourse.bacc as bacc
import concourse.bass as bass
import concourse.mybir as mybir
import concourse.tile as tile
from concourse.tile_rust import add_dep_helper
from concourse.bass_utils import run_bass_kernel_spmd

NCORES = 8
B = 16384
I = 512
H = 512
NG = 7
G = NG * H          # 3584
K2 = I + H          # 1024
P = 128
BS = B // NCORES    # 2048 rows per core
KC = K2 // P        # 8 k-chunks
NB = BS // 512      # 4 batch tiles of 512
NJ = H // P         # 4 h-chunks
NGC = G // P        # 28 gate chunks

BF16 = mybir.dt.bfloat16
F32 = mybir.dt.float32
AF = mybir.ActivationFunctionType
NPBF16 = ml_dtypes.bfloat16

# gate order within each h-chunk j (reference order: i,f,z,o,d,ib,fb)
# position in our per-j block [z, i, f, d, o, ib, fb] -> original index
PERMJ = [2, 0, 1, 4, 3, 5, 6]
GZ, GI, GF, GD, GO, GIB, GFB = 0, 1, 2, 3, 4, 5, 6

TRACE = False
LAST_RESULTS = None

_nc_cache = None


def _build():
    nc = bacc.Bacc("TRN2", target_bir_lowering=False, debug=False)

    xh_d = nc.dram_tensor("xh", [KC, P, BS], BF16, kind="ExternalInput")
    w_d = nc.dram_tensor("w2", [NJ, KC, P, NG * P], BF16, kind="ExternalInput")
    ct_d = nc.dram_tensor("ct", [NJ, P, BS], F32, kind="ExternalInput")
    bias_d = nc.dram_tensor("bias", [P, NGC], F32, kind="ExternalInput")

    h_d = nc.dram_tensor("h", [NJ, P, BS], F32, kind="ExternalOutput")
    c_d = nc.dram_tensor("c", [NJ, P, BS], F32, kind="ExternalOutput")
    cb_d = nc.dram_tensor("cb", [NJ, P, BS], F32, kind="ExternalOutput")
    o_d = nc.dram_tensor("o", [NJ, P, BS], F32, kind="ExternalOutput")
    dr_d = nc.dram_tensor("dr", [NJ, P, BS], F32, kind="ExternalOutput")

    with tile.TileContext(nc) as tc:
        with (
            tc.tile_pool(name="wp", bufs=1) as wp,
            tc.tile_pool(name="bp", bufs=1) as bp,
            tc.tile_pool(name="gp", bufs=4) as gp,
            tc.tile_pool(name="ctp", bufs=3) as ctp,
            tc.tile_pool(name="pp", bufs=8, space=bass.MemorySpace.PSUM) as pp,
        ):
            xh_sb = wp.tile([P, KC, BS], BF16, tag="xh")
            # w laid out j-major: [P, (j, k), 896] so j0's weights arrive
            # first and the PE reaches steady state on ~6MB instead of 11.4
            w_sb = wp.tile([P, NJ * KC, NG * P], BF16, tag="w")
            bias_sb = bp.tile([P, NGC], F32, tag="bias")
            nc.sync.dma_start(w_sb[:, 0, :], w_d[0, 0])
            nc.sync.dma_start(xh_sb[:, 0, :], xh_d[0])
            nc.sync.dma_start(bias_sb[:], bias_d[:])
            for k in range(1, KC):
                nc.sync.dma_start(xh_sb[:, k, :], xh_d[k])
                nc.sync.dma_start(w_sb[:, k, :], w_d[0, k])

            # PE warmup: ~3.5us of dummy matmuls ramps the tensor engine to
            # its top pstate while the first real k-chunks stream in
            warm = bp.tile([P, 512], BF16, tag="warm")
            nc.vector.memset(warm[:], 0.0)
            for wi in range(16):
                wacc = pp.tile([P, 512], F32, tag="acc", name=f"wacc{wi}")
                nc.tensor.matmul(wacc[:], warm[:, 0:P], warm[:],
                                 start=True, stop=True)

            for j in range(NJ):
                # ct tiles for this j (needed by DVE only, lots of lead)
                ctj = ctp.tile([P, BS], F32, tag="ct", name=f"ct{j}")
                nc.sync.dma_start(ctj[:], ct_d[j])
                cts = [ctj[:, n * 512:(n + 1) * 512] for n in range(NB)]
                if j + 1 < NJ:
                    for k in range(KC):
                        nc.sync.dma_start(
                            w_sb[:, (j + 1) * KC + k, :], w_d[j + 1, k])

                gates = [[None] * NB for _ in range(NG)]
                tanh_c = [None] * NB
                last_ln = None
                for gi in range(NG):
                    gc = j * NG + gi
                    gsl = slice(gi * P, (gi + 1) * P)
                    accs = [pp.tile([P, 512], F32, tag="acc",
                                    name=f"acc{gi}_{n}")
                            for n in range(NB)]
                    for k in range(KC):
                        for n in range(NB):
                            nc.tensor.matmul(
                                accs[n][:],
                                w_sb[:, j * KC + k, gsl],
                                xh_sb[:, k, n * 512:(n + 1) * 512],
                                start=(k == 0), stop=(k == KC - 1),
                            )
                    func = AF.Tanh if gi == GZ else AF.Sigmoid
                    scale = -1.0 if gi == GD else 1.0
                    for n in range(NB):
                        t = gp.tile([P, 512], F32, tag=f"g{gi}",
                                    name=f"g{gi}_{n}")
                        drain = nc.scalar.activation(
                            t[:], accs[n][:], func,
                            bias=bias_sb[:, gc:gc + 1], scale=scale)
                        if gi == GO and n == 0 and last_ln is not None:
                            # keep the sigmoid-table drains after this j's
                            # Ln batch: exactly 2 table switches per j
                            add_dep_helper(drain.ins, last_ln.ins,
                                           reason="resume sigmoid after ln")
                        gates[gi][n] = t
                        if gi == GO:
                            nc.sync.dma_start(
                                o_d[j, :, n * 512:(n + 1) * 512], t[:])

                    if gi == GF:
                        # z, i, f drained: c = f*ct + i*z, tanh(c)
                        for n in range(NB):
                            nsl = slice(n * 512, (n + 1) * 512)
                            Z = gates[GZ][n]
                            Ii = gates[GI][n]
                            F = gates[GF][n]
                            nc.vector.tensor_mul(F[:], F[:], cts[n][:])
                            nc.vector.tensor_mul(Ii[:], Ii[:], Z[:])
                            nc.vector.tensor_add(F[:], F[:], Ii[:])  # c
                            nc.sync.dma_start(c_d[j, :, nsl], F[:])
                            tanh_c[n] = nc.scalar.activation(
                                Ii[:], F[:], AF.Tanh)
                    elif gi == GD:
                        # dr = -ln(sigmoid(-wd)), batched behind tanh(c)
                        for n in range(NB):
                            D = gates[GD][n]
                            ln = nc.scalar.activation(D[:], D[:], AF.Ln)
                            add_dep_helper(ln.ins, tanh_c[NB - 1].ins,
                                           reason="ln after main-table acts")
                            last_ln = ln
                            nc.vector.tensor_scalar_mul(D[:], D[:], -1.0)
                            nc.sync.dma_start(
                                dr_d[j, :, n * 512:(n + 1) * 512], D[:])
                    elif gi == GO:
                        # h = o * tanh(c); Z stays live for ib*z below
                        for n in range(NB):
                            nsl = slice(n * 512, (n + 1) * 512)
                            Ii = gates[GI][n]   # holds tanh(c)
                            O = gates[GO][n]
                            nc.vector.tensor_mul(O[:], O[:], Ii[:])
                            nc.sync.dma_start(h_d[j, :, nsl], O[:])
                    elif gi == GIB:
                        # ib*z early so the fb tail is only mul+add
                        for n in range(NB):
                            IB = gates[GIB][n]
                            nc.vector.tensor_mul(IB[:], IB[:],
                                                 gates[GZ][n][:])
                    elif gi == GFB:
                        # c_bar = fb*ct + ib*z
                        for n in range(NB):
                            nsl = slice(n * 512, (n + 1) * 512)
                            IB = gates[GIB][n]
                            FB = gates[GFB][n]
                            nc.vector.tensor_mul(FB[:], FB[:], cts[n][:])
                            nc.vector.tensor_add(FB[:], FB[:], IB[:])
                            nc.sync.dma_start(cb_d[j, :, nsl], FB[:])

    nc.compile()
    return nc


def kernel(x, ht, ct, Wx, bx, Wh, bh):
    global _nc_cache, LAST_RESULTS
    if _nc_cache is None:
        _nc_cache = _build()
    nc = _nc_cache

    x = np.ascontiguousarray(x, dtype=np.float32)
    ht = np.ascontiguousarray(ht, dtype=np.float32)
    ct = np.ascontiguousarray(ct, dtype=np.float32)

    # xh = [x; ht].T as [k, p, b] chunks, bf16
    xh_full = np.empty((K2, B), dtype=NPBF16)
    xh_full[:I, :] = x.T.astype(NPBF16)
    xh_full[I:, :] = ht.T.astype(NPBF16)
    xh_dev = xh_full.reshape(KC, P, B)

    # w2 = [Wx; Wh].T with gate columns permuted j-major:
    # dst chunk (j*7+gi) <- original gate PERMJ[gi], h-chunk j
    WxT = np.asarray(Wx, dtype=np.float32).T   # [512, 3584]
    WhT = np.asarray(Wh, dtype=np.float32).T
    bsum = np.asarray(bx, dtype=np.float32) + np.asarray(bh, dtype=np.float32)
    w2 = np.empty((K2, G), dtype=NPBF16)
    bias_perm = np.empty(G, dtype=np.float32)
    for j in range(NJ):
        for gi, go in enumerate(PERMJ):
            dsl = slice((j * NG + gi) * P, (j * NG + gi + 1) * P)
            ssl = slice(go * H + j * P, go * H + (j + 1) * P)
            w2[:I, dsl] = WxT[:, ssl].astype(NPBF16)
            w2[I:, dsl] = WhT[:, ssl].astype(NPBF16)
            sgn = -1.0 if gi == GD else 1.0
            bias_perm[dsl] = sgn * bsum[ssl]
    # [K2, G] -> [NJ, KC, P, 896]: w_dev[j, k, p, gi*128+c] = w2[k*P+p, (j*7+gi)*128+c]
    w_dev = np.ascontiguousarray(
        w2.reshape(KC, P, NJ, NG * P).transpose(2, 0, 1, 3))
    bias_dev = np.ascontiguousarray(bias_perm.reshape(NGC, P).T)

    in_maps = []
    for cidx in range(NCORES):
        sl = slice(cidx * BS, (cidx + 1) * BS)
        in_maps.append({
            "xh": np.ascontiguousarray(xh_dev[:, :, sl]),
            "w2": w_dev,
            "ct": np.ascontiguousarray(ct[sl].T).reshape(NJ, P, BS),
            "bias": bias_dev,
        })

    res = run_bass_kernel_spmd(nc, in_maps, core_ids=list(range(NCORES)),
                               trace=TRACE)
    LAST_RESULTS = res

    outs = {}
    for name in ("h", "c", "cb", "o", "dr"):
        full = np.concatenate(
            [res.results[cidx][name].reshape(H, BS) for cidx in range(NCORES)],
            axis=1,
        )
        outs[name] = np.ascontiguousarray(full.T)
    return outs["h"], outs["c"], outs["cb"], outs["o"], outs["dr"]
